# revision 5
# baseline (speedup 1.0000x reference)
"""Trainium2 Bass kernel for nn_SWAVerifyChunk1 (8-layer Gemma3-style chunk).

Sharding: tensor-parallel over 8 NeuronCores.
 - core c owns Q head c and KV head c//4 (KV + PLE compute replicated).
 - Wq/Wk/Wv and Wgate/Wup column-parallel; Wo / Wdown row-parallel with an
   fp32 ncfw AllReduce of the [S,2048] partials after each (2 per layer).
 - all matmuls fp32: the unscaled softmax (|scores|~40) amplifies operand
   rounding ~30x through the 8 layers; fp16/bf16 operands measurably miss
   (numpy ablations: all-fp16 -> 5.8e-2 rel, fp32 -> ~1e-4).
 - host preprocessing: shard weights, transpose K caches, pre-shift sliding
   caches, fold w_in/w_pre_ff into weight rows, fold w_q/w_k norm weights
   into the RoPE tables, precompute indicator row-sums.
"""
import sys
sys.path.insert(0, '/opt/trn_rl_repo')
import numpy as np

S = 128; HID = 2048; NH = 8; NKV = 2; HD = 256; FF = 4096
PLD = 256; NLAYERS = 8; WIN = 512; FULL_LEN = 2048
FULL_LAYER = 5
PLE_PROJ_SCALE = HID ** -0.5
RSQRT2 = 0.7071067811865476
NCORES = 8
P = 128
NCH = HID // P           # 16 hidden-dim K-chunks
FFS = FF // NCORES       # 512 FF cols per core
EPS = 1e-6

_BUILD_CACHE = {}


def _build(ones_posts: bool, ones_lscal: bool):
    key = (ones_posts, ones_lscal)
    if key in _BUILD_CACHE:
        return _BUILD_CACHE[key]

    import contextlib
    import concourse.bass as bass
    import concourse.tile as tile
    from concourse import bacc, mybir
    from concourse.masks import make_identity

    f32 = mybir.dt.float32
    AF = mybir.ActivationFunctionType
    ALU = mybir.AluOpType
    AX = mybir.AxisListType

    nc = bacc.Bacc("TRN2", target_bir_lowering=False, debug=False,
                   num_devices=NCORES)

    def inp(name, shape):
        return nc.dram_tensor(name, shape, f32, kind="ExternalInput").ap()

    hs0_d = inp("hs0", [S, HID])
    hT0_d = inp("hT0", [HID, S])
    raw2_d = inp("raw2", [S, NLAYERS * PLD])
    wple_d = inp("wple", [1, PLD])
    wqkv_d = inp("wqkv", [NLAYERS, HID, 3 * HD])
    wo_d = inp("wo", [NLAYERS, HD, HID])
    wgu_d = inp("wgu", [NLAYERS, HID, 2 * FFS])
    wdn_d = inp("wdn", [NLAYERS, FFS, HID])
    wpg_d = inp("wpg", [NLAYERS, HID, PLD])
    wpp_d = inp("wpp", [NLAYERS, PLD, HID])
    wmp_d = inp("wmp", [HID, NLAYERS * PLD])
    ropes_d = inp("ropes", [NLAYERS, 4, S, HD])
    ktc_d = inp("ktc", [7, 2, P, WIN - S])
    vc_d = inp("vc", [7, WIN - S, HD])
    kftc_d = inp("kftc", [2, P, FULL_LEN])
    vfc_d = inp("vfc", [FULL_LEN, HD])
    indT_d = inp("indT", [S, FULL_LEN])
    cm1m_row_d = inp("cm1m_row", [1, FULL_LEN])
    cm1m_col_d = inp("cm1m_col", [FULL_LEN // P, P])
    maskf_d = inp("maskf", [S, FULL_LEN])
    masks_d = inp("masks", [S, WIN])
    wposts_d = inp("wposts", [NLAYERS, 3 * HID])
    lscal_d = inp("lscal", [1, NLAYERS])

    out_hs = nc.dram_tensor("out_hs", [S, HID], f32, kind="ExternalOutput").ap()
    out_k = nc.dram_tensor("out_k", [NLAYERS, S, HD], f32, kind="ExternalOutput").ap()
    out_v = nc.dram_tensor("out_v", [NLAYERS, S, HD], f32, kind="ExternalOutput").ap()

    with tile.TileContext(nc) as tc:
        with contextlib.ExitStack() as ctx:
            ep = ctx.enter_context
            persist = ep(tc.tile_pool(name="persist", bufs=1))
            wpool = ep(tc.tile_pool(name="wpool", bufs=2))      # 2MB granules
            big = ep(tc.tile_pool(name="big", bufs=1))          # ~1-2MB tiles
            roll = ep(tc.tile_pool(name="roll", bufs=2))        # residual chain
            sm = ep(tc.tile_pool(name="sm", bufs=1))            # <=0.25MB tiles
            scal = ep(tc.tile_pool(name="scal", bufs=4))        # [S,1] scalars
            rope_p = ep(tc.tile_pool(name="rope", bufs=2))
            pt = ep(tc.tile_pool(name="pt", bufs=2, space="PSUM"))   # 1 bank
            pm = ep(tc.tile_pool(name="pm", bufs=1, space="PSUM"))   # 2 banks
            pb = ep(tc.tile_pool(name="pb", bufs=1, space="PSUM"))   # 4 banks
            dram = ep(tc.tile_pool(name="dram", bufs=4, space="DRAM"))

            # ---- constants / persistent
            ident = persist.tile([P, P], f32)
            make_identity(nc, ident)
            wple_b = persist.tile([P, PLD], f32)
            nc.sync.dma_start(wple_b, wple_d.to_broadcast((P, PLD)))
            if not ones_lscal:
                lsc_b = persist.tile([P, NLAYERS], f32)
                nc.sync.dma_start(lsc_b, lscal_d.to_broadcast((P, NLAYERS)))
            hs = roll.tile([S, HID], f32, tag="hsroll")
            nc.sync.dma_start(hs[:], hs0_d)
            hT0 = persist.tile([P, NCH, S], f32)
            nc.sync.dma_start(hT0, hT0_d.rearrange("(c p) s -> p c s", p=P))
            raw2 = persist.tile([S, NLAYERS * PLD], f32)
            nc.sync.dma_start(raw2, raw2_d)
            ple = persist.tile([S, NLAYERS * PLD], f32)
            mask_s = persist.tile([S, WIN], f32)
            nc.sync.dma_start(mask_s, masks_d)

            eps_t = persist.tile([S, 1], f32)
            nc.vector.memset(eps_t, EPS)

            # warmup collective: rendezvous cores under the prologue DMAs
            cw_in = dram.tile([P, P], f32, tag="cw")
            cw_out = dram.tile([P, P], f32, tag="cw")
            nc.sync.dma_start(cw_in[:], ident)
            nc.gpsimd.collective_compute(
                "AllReduce", ALU.add, ins=[cw_in.opt()], outs=[cw_out.opt()],
                replica_groups=[list(range(NCORES))])

            def rsqrt_meansq(ss, n, tag):
                r = scal.tile([S, 1], f32, tag=tag)
                nc.scalar.activation(r, ss, AF.Sqrt, bias=eps_t, scale=1.0 / n)
                nc.vector.reciprocal(r, r)
                return r

            def sumsq_big(src, tag):
                """[S,1] sum(src^2) along free dim; src up to [S,HID]."""
                ss = scal.tile([S, 1], f32, tag=tag)
                sq = big.tile([S, HID], f32, tag="sq")
                nc.scalar.activation(sq[:, :src.shape[-1]], src, AF.Square,
                                     accum_out=ss)
                return ss

            def sumsq_sm(src, tag):
                ss = scal.tile([S, 1], f32, tag=tag)
                sq = sm.tile([S, HD], f32, tag="sqs")
                nc.scalar.activation(sq[:, :src.shape[-1]], src, AF.Square,
                                     accum_out=ss)
                return ss

            def transpose_into(src, nchunks, dst, scale=None, dst_off=0,
                               col_off=0):
                """PE-transpose src [S, nchunks*128] -> dst[:, dst_off+c, :]
                (dst free-dim layout [.., chunk, S]); optional per-source-row
                scale [S,1] fused via diag-matmul."""
                dg = None
                if scale is not None:
                    dg = sm.tile([P, P], f32, tag="diag")
                    nc.vector.tensor_scalar_mul(dg, ident, scale)
                for c in range(nchunks):
                    ps = pt.tile([P, P], f32, tag="tp_ps")
                    src_sl = src[:, col_off + c * P:col_off + (c + 1) * P]
                    if dg is not None:
                        nc.tensor.matmul(ps, src_sl, dg, start=True, stop=True)
                    else:
                        nc.tensor.transpose(ps, src_sl, ident)
                    if c % 2 == 0:
                        nc.vector.tensor_copy(dst[:, dst_off + c, :], ps)
                    else:
                        nc.scalar.copy(dst[:, dst_off + c, :], ps)

            # ---- PLE pre-pass chunk l (emitted under layer l-1's AR windows)
            def ple_chunk(l):
                wmp_sb = wpool.tile([P, NCH, PLD], f32, tag="w")
                nc.sync.dma_start(
                    wmp_sb,
                    wmp_d[:, l * PLD:(l + 1) * PLD]
                    .rearrange("(c p) n -> p c n", p=P))
                pp = pt.tile([S, PLD], f32, tag="tp_ps")
                for c in range(NCH):
                    nc.tensor.matmul(pp, hT0[:, c, :], wmp_sb[:, c, :],
                                     start=(c == 0), stop=(c == NCH - 1))
                ss = sumsq_sm(pp, "ss_ple")
                rs = rsqrt_meansq(ss, PLD, "rs_ple")
                t = sm.tile([S, PLD], f32, tag="ple_t")
                nc.vector.scalar_tensor_tensor(t, pp, rs, wple_b,
                                               op0=ALU.mult, op1=ALU.mult)
                nc.vector.tensor_add(ple[:, l * PLD:(l + 1) * PLD], t,
                                     raw2[:, l * PLD:(l + 1) * PLD])

            ple_chunk(0)

            def matmul_colpar(lhsT, w_dram, ncols, psum, nslices):
                """psum[S, ncols] += lhsT.T @ W  streaming W in granules of
                4 K-chunks; nslices = list of (lo, hi) N-slices (<=512)."""
                GK = 4
                for g0 in range(0, NCH, GK):
                    gsz = min(GK, NCH - g0)
                    w_sb = wpool.tile([P, GK, ncols], f32, tag="w")
                    nc.sync.dma_start(
                        w_sb[:, :gsz, :],
                        w_dram[g0 * P:(g0 + gsz) * P, :]
                        .rearrange("(c p) n -> p c n", p=P))
                    for ci in range(gsz):
                        c = g0 + ci
                        for (lo, hi) in nslices:
                            nc.tensor.matmul(psum[:, lo:hi], lhsT[:, c, :],
                                             w_sb[:, ci, lo:hi],
                                             start=(c == 0),
                                             stop=(c == NCH - 1))

            def matmul_rowpar(lhsT, w_dram, nk, psum):
                """psum[S, HID] = lhsT[P, nk, S].T @ W[nk*128, HID]."""
                GK = 2
                for g0 in range(0, nk, GK):
                    gsz = min(GK, nk - g0)
                    w_sb = wpool.tile([P, GK, HID], f32, tag="w")
                    nc.sync.dma_start(
                        w_sb[:, :gsz, :],
                        w_dram[g0 * P:(g0 + gsz) * P, :]
                        .rearrange("(c p) n -> p c n", p=P))
                    for ci in range(gsz):
                        c = g0 + ci
                        for n in range(HID // 512):
                            nc.tensor.matmul(psum[:, n * 512:(n + 1) * 512],
                                             lhsT[:, c, :],
                                             w_sb[:, ci, n * 512:(n + 1) * 512],
                                             start=(c == 0), stop=(c == nk - 1))

            def allreduce(src_ps):
                """fp32 AllReduce of a [S, HID] psum partial; returns sbuf tile."""
                y = big.tile([S, HID], f32, tag="y_loc")
                nc.scalar.copy(y, src_ps)
                a_in = dram.tile([S, HID], f32, tag="ar_in")
                a_out = dram.tile([S, HID], f32, tag="ar_out")
                nc.sync.dma_start(a_in[:], y)
                nc.gpsimd.collective_compute(
                    "AllReduce", ALU.add, ins=[a_in.opt()], outs=[a_out.opt()],
                    replica_groups=[list(range(NCORES))])
                yr = big.tile([S, HID], f32, tag="y_red")
                nc.sync.dma_start(yr[:], a_out[:])
                return yr

            def post_norm_residual(y, res, wpost_sl, nfeat):
                """res + vnorm(y)*w ; y may be PSUM or SBUF."""
                ss = sumsq_big(y, "ss_pn")
                rs = rsqrt_meansq(ss, nfeat, "rs_pn")
                out = roll.tile([S, HID], f32, tag="hsroll")
                if ones_posts:
                    nc.vector.scalar_tensor_tensor(out, y, rs, res,
                                                   op0=ALU.mult, op1=ALU.add)
                else:
                    t = big.tile([S, HID], f32, tag="pn_t")
                    nc.vector.scalar_tensor_tensor(t, y, rs, wpost_sl,
                                                   op0=ALU.mult, op1=ALU.mult)
                    nc.vector.tensor_add(out, t, res)
                return out

            # =================== layers ===================
            si = 0
            for i in range(NLAYERS):
                is_full = (i == FULL_LAYER)
                L = FULL_LEN if is_full else WIN
                LCH = L // P

                wposts_sb = None
                if not ones_posts:
                    wposts_sb = big.tile([P, 3 * HID], f32, tag="wposts")
                    nc.sync.dma_start(
                        wposts_sb, wposts_d[i:i + 1, :].to_broadcast((P, 3 * HID)))

                # ---- input norm -> normalized transposed activations
                ss_in = sumsq_big(hs, "ss_in")
                rs_in = rsqrt_meansq(ss_in, HID, "rs_in")
                hT = big.tile([P, NCH, S], f32, tag="hT")
                transpose_into(hs, NCH, hT, scale=rs_in)

                # ---- QKV
                qkv = pm.tile([S, 3 * HD], f32, tag="pm")
                matmul_colpar(hT, wqkv_d[i], 3 * HD, qkv,
                              [(0, 512), (512, 768)])

                rope_sb = rope_p.tile([S, 4, HD], f32, tag="rope")
                nc.sync.dma_start(rope_sb,
                                  ropes_d[i].rearrange("f s d -> s f d"))

                def norm_rope(src, cos_t, sin_t, tag):
                    ssq = sumsq_sm(src, "ss_" + tag)
                    r = rsqrt_meansq(ssq, HD, "rs_" + tag)
                    xn = sm.tile([S, HD], f32, tag="xn")
                    nc.vector.tensor_scalar_mul(xn, src, r)
                    H2 = HD // 2
                    t1 = sm.tile([S, HD], f32, tag="rt1")
                    nc.vector.tensor_mul(t1, xn, cos_t)
                    t2 = sm.tile([S, HD], f32, tag="rt2")
                    nc.vector.tensor_mul(t2[:, 0:H2], xn[:, H2:HD],
                                         sin_t[:, 0:H2])
                    nc.vector.tensor_mul(t2[:, H2:HD], xn[:, 0:H2],
                                         sin_t[:, H2:HD])
                    xr = sm.tile([S, HD], f32, tag="xr_" + tag)
                    nc.vector.tensor_sub(xr[:, 0:H2], t1[:, 0:H2], t2[:, 0:H2])
                    nc.vector.tensor_add(xr[:, H2:HD], t1[:, H2:HD],
                                         t2[:, H2:HD])
                    return xr

                q_r = norm_rope(qkv[:, 0:HD], rope_sb[:, 0, :],
                                rope_sb[:, 1, :], "q")
                k_r = norm_rope(qkv[:, HD:2 * HD], rope_sb[:, 2, :],
                                rope_sb[:, 3, :], "k")
                nc.sync.dma_start(out_k[i], k_r)
                ssv = sumsq_sm(qkv[:, 2 * HD:3 * HD], "ss_v")
                rv = rsqrt_meansq(ssv, HD, "rs_v")
                v_t = sm.tile([S, HD], f32, tag="v")
                nc.vector.tensor_scalar_mul(v_t, qkv[:, 2 * HD:3 * HD], rv)
                nc.sync.dma_start(out_v[i], v_t)

                qT = sm.tile([P, 2, S], f32, tag="qT")
                transpose_into(q_r, 2, qT)

                # ---- K^T [P, 2, L] and V [P, LCH, HD]
                KT = big.tile([P, 2, FULL_LEN], f32, tag="KT")
                VA = big.tile([P, FULL_LEN // P, HD], f32, tag="VA")
                if not is_full:
                    nc.sync.dma_start(KT[:, :, 0:WIN - S],
                                      ktc_d[si].rearrange("c p l -> p c l"))
                    nc.sync.dma_start(VA[:, 0:LCH - 1, :],
                                      vc_d[si].rearrange("(c p) d -> p c d", p=P))
                    for c in range(2):
                        psk = pt.tile([P, P], f32, tag="tp_ps")
                        nc.tensor.transpose(psk, k_r[:, c * P:(c + 1) * P], ident)
                        nc.vector.tensor_copy(KT[:, c, WIN - S:WIN], psk)
                    nc.scalar.copy(VA[:, LCH - 1, :], v_t)
                    mask_t = mask_s
                else:
                    indT_sb = persist.tile([S, FULL_LEN], f32)
                    nc.sync.dma_start(indT_sb, indT_d)
                    bc = big.tile([P, FULL_LEN], f32, tag="y_loc")
                    nc.sync.dma_start(bc, cm1m_row_d.to_broadcast((P, FULL_LEN)))
                    cmc = persist.tile([P, FULL_LEN // P], f32)
                    nc.sync.dma_start(cmc, cm1m_col_d.rearrange("c p -> p c"))
                    nc.sync.dma_start(KT[:, :, :],
                                      kftc_d.rearrange("c p l -> p c l"))
                    nc.sync.dma_start(VA[:, :, :],
                                      vfc_d.rearrange("(c p) d -> p c d", p=P))
                    for c in range(2):
                        ksc = pb.tile([S, HID], f32, tag="pb")
                        for n in range(FULL_LEN // 512):
                            nc.tensor.matmul(ksc[:, n * 512:(n + 1) * 512],
                                             k_r[:, c * P:(c + 1) * P],
                                             indT_sb[:, n * 512:(n + 1) * 512],
                                             start=True, stop=True)
                        nc.vector.tensor_mul(KT[:, c, :], KT[:, c, :], bc)
                        nc.vector.tensor_add(KT[:, c, :], KT[:, c, :], ksc)
                    for tch in range(FULL_LEN // P):
                        vsc = pt.tile([S, HD], f32, tag="tp_ps")
                        nc.tensor.matmul(vsc, indT_sb[:, tch * P:(tch + 1) * P],
                                         v_t, start=True, stop=True)
                        nc.vector.scalar_tensor_tensor(
                            VA[:, tch, :], VA[:, tch, :], cmc[:, tch:tch + 1],
                            vsc, op0=ALU.mult, op1=ALU.add)
                    mask_t = persist.tile([S, FULL_LEN], f32)
                    nc.sync.dma_start(mask_t, maskf_d)

                # ---- scores + softmax (in fp32)
                sc_ps = (pb if is_full else pm).tile(
                    [S, L], f32, tag="pb" if is_full else "pm")
                for c in range(2):
                    for n in range(L // 512):
                        nc.tensor.matmul(sc_ps[:, n * 512:(n + 1) * 512],
                                         qT[:, c, :],
                                         KT[:, c, n * 512:(n + 1) * 512],
                                         start=(c == 0), stop=(c == 1))
                scores = big.tile([S, FULL_LEN], f32, tag="scores")
                nc.vector.tensor_add(scores[:, 0:L], sc_ps, mask_t[:, 0:L])
                nmax = scal.tile([S, 1], f32, tag="nmax")
                nc.vector.tensor_reduce(nmax, scores[:, 0:L], AX.X, ALU.max,
                                        negate=True)
                den = scal.tile([S, 1], f32, tag="den")
                nc.scalar.activation(scores[:, 0:L], scores[:, 0:L], AF.Exp,
                                     bias=nmax, accum_out=den)
                rden = scal.tile([S, 1], f32, tag="rden")
                nc.vector.reciprocal(rden, den)

                # ---- attn = expT.T @ V, row-scaled by 1/den
                pT = big.tile([P, FULL_LEN // P, S], f32, tag="pT")
                transpose_into(scores, LCH, pT)
                at_ps = pt.tile([S, HD], f32, tag="tp_ps")
                for tch in range(LCH):
                    nc.tensor.matmul(at_ps, pT[:, tch, :], VA[:, tch, :],
                                     start=(tch == 0), stop=(tch == LCH - 1))
                attn = sm.tile([S, HD], f32, tag="attn")
                nc.vector.tensor_scalar_mul(attn, at_ps, rden)
                aT = sm.tile([P, 2, S], f32, tag="aT")
                transpose_into(attn, 2, aT)

                # ---- Wo row-parallel + AR1
                wo_ps = pb.tile([S, HID], f32, tag="pb")
                matmul_rowpar(aT, wo_d[i], 2, wo_ps)
                y1r = allreduce(wo_ps)
                if i + 1 < NLAYERS:
                    ple_chunk(i + 1)
                hs_a = post_norm_residual(
                    y1r, hs, None if ones_posts else wposts_sb[:, 0:HID], HID)

                # ---- MLP
                ss2 = sumsq_big(hs_a, "ss2")
                rs2 = rsqrt_meansq(ss2, HID, "rs2")
                h2T = big.tile([P, NCH, S], f32, tag="hT")
                transpose_into(hs_a, NCH, h2T, scale=rs2)
                gu_ps = pm.tile([S, 2 * FFS], f32, tag="pm")
                matmul_colpar(h2T, wgu_d[i], 2 * FFS, gu_ps,
                              [(0, FFS), (FFS, 2 * FFS)])
                gel = sm.tile([S, FFS], f32, tag="gel")
                nc.scalar.activation(gel, gu_ps[:, 0:FFS], AF.Gelu_apprx_tanh)
                mid = sm.tile([S, FFS], f32, tag="mid")
                nc.vector.tensor_mul(mid, gel, gu_ps[:, FFS:2 * FFS])
                midT = sm.tile([P, FFS // P, S], f32, tag="midT")
                transpose_into(mid, FFS // P, midT)
                dn_ps = pb.tile([S, HID], f32, tag="pb")
                matmul_rowpar(midT, wdn_d[i], FFS // P, dn_ps)
                y2r = allreduce(dn_ps)
                hs_b = post_norm_residual(
                    y2r, hs_a, None if ones_posts else wposts_sb[:, HID:2 * HID], HID)

                # ---- PLE block (replicated)
                hbT = big.tile([P, NCH, S], f32, tag="hT")
                transpose_into(hs_b, NCH, hbT)
                pg_ps = pt.tile([S, PLD], f32, tag="tp_ps")
                matmul_colpar(hbT, wpg_d[i], PLD, pg_ps, [(0, PLD)])
                gel2 = sm.tile([S, PLD], f32, tag="gel2")
                nc.scalar.activation(gel2, pg_ps, AF.Gelu_apprx_tanh)
                g_t = sm.tile([S, PLD], f32, tag="g")
                nc.vector.tensor_mul(g_t, gel2, ple[:, i * PLD:(i + 1) * PLD])
                gT = sm.tile([P, 2, S], f32, tag="gT")
                transpose_into(g_t, 2, gT)
                pp_ps = pb.tile([S, HID], f32, tag="pb")
                matmul_rowpar(gT, wpp_d[i], 2, pp_ps)
                hs_new = post_norm_residual(
                    pp_ps, hs_b, None if ones_posts else wposts_sb[:, 2 * HID:3 * HID], HID)
                if not ones_lscal:
                    nc.scalar.activation(hs_new, hs_new, AF.Copy,
                                         scale=lsc_b[:, i:i + 1])
                hs = hs_new
                if not is_full:
                    si += 1

            nc.sync.dma_start(out_hs, hs[:])

    nc.compile()
    _BUILD_CACHE[key] = nc
    return nc


def _shard_inputs(inp):
    g = lambda n: np.asarray(inp[n], np.float32)
    hs0 = g('hidden_states')[0]
    raw2 = g('per_layer_raw')[0][:, :NLAYERS * PLD] * RSQRT2
    wple = (g('w_ple_norm') * RSQRT2).reshape(1, PLD)
    Wmp = np.ascontiguousarray(
        g('Wple_model_proj')[:, :NLAYERS * PLD]) * PLE_PROJ_SCALE
    cos_s, sin_s = g('cos_s')[0], g('sin_s')[0]
    cos_f, sin_f = g('cos_f')[0], g('sin_f')[0]
    w_in = g('w_in_ln'); w_pre = g('w_pre_ff_ln')
    Wq, Wk, Wv, Wo = g('Wq'), g('Wk'), g('Wv'), g('Wo')
    Wgate, Wup, Wdown = g('Wgate'), g('Wup'), g('Wdown')
    ind = g('update_indicator')[0, 0]
    cm1m = 1.0 - ind.sum(-1)

    H2 = HD // 2
    ropes = np.empty((NLAYERS, 4, S, HD), np.float32)
    for i in range(NLAYERS):
        cos, sin = (cos_f, sin_f) if i == FULL_LAYER else (cos_s, sin_s)
        wq = g('w_q_norm')[i]; wk = g('w_k_norm')[i]
        wq_sw = np.concatenate([wq[H2:], wq[:H2]])
        wk_sw = np.concatenate([wk[H2:], wk[:H2]])
        ropes[i, 0] = cos * wq; ropes[i, 1] = sin * wq_sw
        ropes[i, 2] = cos * wk; ropes[i, 3] = sin * wk_sw

    wposts = np.ascontiguousarray(np.stack(
        [g('w_post_attn_ln'), g('w_post_ff_ln'), g('w_post_ple_ln')],
        1).reshape(NLAYERS, 3 * HID))
    lscal = np.ascontiguousarray(g('layer_scalar').reshape(1, NLAYERS))

    common = dict(
        hs0=np.ascontiguousarray(hs0),
        hT0=np.ascontiguousarray(hs0.T),
        raw2=np.ascontiguousarray(raw2),
        wple=np.ascontiguousarray(wple),
        wmp=np.ascontiguousarray(Wmp),
        ropes=ropes,
        indT=np.ascontiguousarray(ind.T),
        cm1m_row=np.ascontiguousarray(cm1m.reshape(1, FULL_LEN)),
        cm1m_col=np.ascontiguousarray(cm1m.reshape(FULL_LEN // P, P)),
        maskf=np.ascontiguousarray(g('causal_mask_full')[0, 0]),
        masks=np.ascontiguousarray(g('causal_mask_sliding')[0, 0]),
        wposts=wposts, lscal=lscal,
        wpg=np.ascontiguousarray(g('Wple_gate')),
        wpp=np.ascontiguousarray(g('Wple_proj')),
    )

    Ks, Vs = g('K_sliding'), g('V_sliding')
    Kf, Vf = g('K_full')[0], g('V_full')[0]

    in_maps = []
    for c in range(NCORES):
        kvh = c // (NH // NKV)
        wqkv = np.concatenate([
            Wq[:, :, c * HD:(c + 1) * HD],
            Wk[:, :, kvh * HD:(kvh + 1) * HD],
            Wv[:, :, kvh * HD:(kvh + 1) * HD]], axis=2) * w_in[:, :, None]
        wgu = np.concatenate([
            Wgate[:, :, c * FFS:(c + 1) * FFS],
            Wup[:, :, c * FFS:(c + 1) * FFS]], axis=2) * w_pre[:, :, None]
        m = dict(common)
        m.update(
            wqkv=np.ascontiguousarray(wqkv),
            wo=np.ascontiguousarray(Wo[:, c * HD:(c + 1) * HD, :]),
            wgu=np.ascontiguousarray(wgu),
            wdn=np.ascontiguousarray(Wdown[:, c * FFS:(c + 1) * FFS, :]),
            ktc=np.ascontiguousarray(
                Ks[:, kvh, S:, :].transpose(0, 2, 1)
                .reshape(7, 2, P, WIN - S)),
            vc=np.ascontiguousarray(Vs[:, kvh, S:, :]),
            kftc=np.ascontiguousarray(Kf[kvh].T.reshape(2, P, FULL_LEN)),
            vfc=np.ascontiguousarray(Vf[kvh]),
        )
        in_maps.append(m)
    return in_maps


def kernel(**inputs):
    from concourse.bass_utils import run_bass_kernel_spmd
    wposts = np.stack([np.asarray(inputs['w_post_attn_ln'], np.float32),
                       np.asarray(inputs['w_post_ff_ln'], np.float32),
                       np.asarray(inputs['w_post_ple_ln'], np.float32)])
    ones_posts = bool(np.all(wposts == 1.0))
    ones_lscal = bool(
        np.all(np.asarray(inputs['layer_scalar'], np.float32) == 1.0))
    nc = _build(ones_posts, ones_lscal)
    in_maps = _shard_inputs(inputs)
    res = run_bass_kernel_spmd(nc, in_maps, list(range(NCORES))).results
    hs = res[0]["out_hs"][None]                             # [1, S, HID]
    new_ks = np.stack([res[0]["out_k"], res[4]["out_k"]], 1)[:, None]
    new_vs = np.stack([res[0]["out_v"], res[4]["out_v"]], 1)[:, None]
    return hs, new_ks, new_vs
